# revision 19
# baseline (speedup 1.0000x reference)
"""HSAUltraLongBlock chunking layer for 8 TRN2 NeuronCores.

Math (per reference): x [N=2, T=8192, d=2048] f32 ->
  enc   = rmsnorm(x) (per-token, chunk-reshape is a no-op for the norm)
  mem_k = enc @ Wk.T          -> [N, T, 8, 64]
  mem_v = enc @ Wv.T          -> [N, T, 8, 64]
  lmk   = mean_chunk(enc) @ Wl.T -> [N, 64, 8, 64]
  returns (x, mem_k, mem_v, lmk)

Sharding: data-parallel over flattened (N*T) tokens -- 16384 tokens / 8 cores
= 2048 tokens (16 chunks of 128) per core.  Each core holds the full
(host-pre-transposed, bf16) weights.  This halves per-core DMA vs. the
batch-DP x head-TP split (each core reads 1/8 of x instead of 1/2) with
identical FLOPs per core.

Per-core pipeline (token tile = 128 tokens = exactly one chunk):
  1. DMA x tile [128 tok, 2048 d] bf16 (host-cast).
  2. ScalarE: Square w/ accum_out -> sumsq[128,1]; Sqrt(sumsq/2048+eps);
     VectorE reciprocal -> s = rsqrt(mean(x^2)+eps).  The per-token scale s
     commutes through the linear projections, so matmuls consume RAW x and
     s is applied to the matmul OUTPUT (per-partition scalar there).
  3. PE: transpose x tile into xT chunks [128 d, 128 tok] (PSUM->SBUF).
  4. PE: mem_k/v tile = xT_chunk.T @ wkT/wvT d-tiles, accumulated over d in
     PSUM -> [128 tok, 512]; VectorE applies s during PSUM->SBUF; DMA out.
  5. Landmarks: chunk-sum(s*x) via matmul with s as stationary -> [1, 2048]
     per chunk; stack (bf16) -> [16, 2048]; PE-transpose -> [128 d, 16 c]
     tiles; project against wlT (pre-scaled by 1/128 on host) -> [16, 512].
"""

import numpy as np
import ml_dtypes

import concourse.bass as bass
import concourse.bacc as bacc
import concourse.mybir as mybir
from concourse.tile import TileContext
from concourse import masks
from concourse.bass_utils import run_bass_kernel_spmd

N_CORES = 8
N_BATCH = 2
T_FULL = 8192
D = 2048
KV = 512
TOK = (N_BATCH * T_FULL) // N_CORES  # 2048 tokens per core
NT = TOK // 128                      # 16 token tiles (= chunks) per core
ND = D // 128                        # 16 d tiles
EPS = 1e-6

F32 = mybir.dt.float32
BF16 = mybir.dt.bfloat16

# stash for test.py introspection (exec_time_ns etc.)
LAST_RESULTS = None
_NC_CACHE = None


def _build_nc():
    nc = bacc.Bacc(None, target_bir_lowering=False)

    x_d = nc.declare_dram_parameter("x", [TOK, D], BF16, isOutput=False)
    wk_d = nc.declare_dram_parameter("wkT", [D, KV], BF16, isOutput=False)
    wv_d = nc.declare_dram_parameter("wvT", [D, KV], BF16, isOutput=False)
    wl_d = nc.declare_dram_parameter("wlT", [D, KV], BF16, isOutput=False)
    mk_d = nc.declare_dram_parameter("mem_k", [TOK, KV], F32, isOutput=True)
    mv_d = nc.declare_dram_parameter("mem_v", [TOK, KV], F32, isOutput=True)
    lm_d = nc.declare_dram_parameter("lmk", [NT, KV], F32, isOutput=True)

    AF = mybir.ActivationFunctionType

    with TileContext(nc) as tc:
        with (
            tc.tile_pool(name="const", bufs=1) as cpool,
            tc.tile_pool(name="wts", bufs=1) as wpool,
            tc.tile_pool(name="x", bufs=3) as xpool,
            tc.tile_pool(name="sq", bufs=1) as sqpool,
            tc.tile_pool(name="stat", bufs=3) as spool,
            tc.tile_pool(name="xt", bufs=2) as xtpool,
            tc.tile_pool(name="out", bufs=3) as opool,
            tc.tile_pool(name="lm", bufs=1) as lmpool,
            tc.tile_pool(name="ps_xt", bufs=2, space="PSUM") as ps_xt,
            tc.tile_pool(name="ps_ok", bufs=2, space="PSUM") as ps_ok,
            tc.tile_pool(name="ps_ov", bufs=1, space="PSUM") as ps_ov,
            tc.tile_pool(name="ps_mean", bufs=2, space="PSUM") as ps_mean,
            tc.tile_pool(name="ps_m", bufs=1, space="PSUM") as ps_m,
        ):
            ident = cpool.tile([128, 128], BF16)
            masks.make_identity(nc, ident[:])
            ident_f = cpool.tile([128, 128], F32)
            masks.make_identity(nc, ident_f[:])
            eps_ap = cpool.tile([128, 1], F32)
            nc.vector.memset(eps_ap[:], EPS)

            # Weights: [D, KV] viewed as [128, ND, KV]; per-partition rows are
            # KV*2B=1KB contiguous -> good DMA.
            wk_sb = wpool.tile([128, ND, KV], BF16)
            wv_sb = wpool.tile([128, ND, KV], BF16)
            wl_sb = wpool.tile([128, ND, KV], BF16)
            for wd, wsb in ((wk_d, wk_sb), (wv_d, wv_sb), (wl_d, wl_sb)):
                nc.sync.dma_start(
                    out=wsb[:], in_=wd.rearrange("(kt p) n -> p kt n", p=128)
                )

            # landmark chunk-means (scaled sums).  Compute engines may only
            # write partition 0 here, so chunks stage along partition-0's free
            # dim and a SBUF->SBUF DMA scatters them across partitions.
            mean_stage = lmpool.tile([1, NT * D], BF16)
            mean_sb = lmpool.tile([NT, D], BF16)

            x_t3 = x_d.rearrange("(nt p) d -> nt p d", p=128)
            mk_t3 = mk_d.rearrange("(nt p) n -> nt p n", p=128)
            mv_t3 = mv_d.rearrange("(nt p) n -> nt p n", p=128)

            for tt in range(NT):
                x_t = xpool.tile([128, D], BF16, tag="x")
                nc.sync.dma_start(out=x_t[:], in_=x_t3[tt])

                # --- rmsnorm scale s = 1/sqrt(mean(x^2)+eps) ---
                sq = sqpool.tile([128, D], BF16, tag="sq")  # discarded
                ss = spool.tile([128, 1], F32, tag="ss")
                nc.scalar.activation(
                    out=sq[:], in_=x_t[:], func=AF.Square, accum_out=ss[:]
                )
                s_rt = spool.tile([128, 1], F32, tag="srt")
                nc.scalar.activation(
                    out=s_rt[:], in_=ss[:], func=AF.Sqrt, scale=1.0 / D,
                    bias=eps_ap[:],
                )
                s_f = spool.tile([128, 1], F32, tag="sf")
                nc.vector.reciprocal(out=s_f[:], in_=s_rt[:])
                s_b = spool.tile([128, 1], BF16, tag="sb")
                nc.vector.tensor_copy(out=s_b[:], in_=s_f[:])

                # --- transpose x tile -> xT chunks [128 d, 128 tok] ---
                xT = xtpool.tile([128, ND, 128], BF16, tag="xt")
                for j4 in range(4):
                    xt_ps = ps_xt.tile([128, 512], BF16, tag="xtp")
                    for jj in range(4):
                        c = j4 * 4 + jj
                        nc.tensor.transpose(
                            out=xt_ps[:, jj * 128 : (jj + 1) * 128],
                            in_=x_t[:, c * 128 : (c + 1) * 128],
                            identity=ident[:],
                        )
                    nc.vector.tensor_copy(
                        out=xT[:, j4 * 4 : (j4 + 1) * 4, :], in_=xt_ps[:]
                    )

                # --- K/V projections, accumulate over d ---
                ok_ps = ps_ok.tile([128, KV], F32, tag="ok")
                ov_ps = ps_ov.tile([128, KV], F32, tag="ov")
                for kt in range(ND):
                    st, sp = kt == 0, kt == ND - 1
                    nc.tensor.matmul(
                        ok_ps[:], lhsT=xT[:, kt, :], rhs=wk_sb[:, kt, :],
                        start=st, stop=sp,
                    )
                    nc.tensor.matmul(
                        ov_ps[:], lhsT=xT[:, kt, :], rhs=wv_sb[:, kt, :],
                        start=st, stop=sp,
                    )
                mk_sb = opool.tile([128, KV], F32, tag="mk")
                nc.vector.tensor_scalar_mul(mk_sb[:], ok_ps[:], s_f[:])
                nc.sync.dma_start(out=mk_t3[tt], in_=mk_sb[:])
                mv_sb = opool.tile([128, KV], F32, tag="mv")
                nc.vector.tensor_scalar_mul(mv_sb[:], ov_ps[:], s_f[:])
                nc.sync.dma_start(out=mv_t3[tt], in_=mv_sb[:])

                # --- landmark scaled chunk-sum: sum_t s_t * x[t, :] ---
                for seg in range(4):
                    mn_ps = ps_mean.tile([1, 512], F32, tag="mean")
                    nc.tensor.matmul(
                        mn_ps[:], lhsT=s_b[:],
                        rhs=x_t[:, seg * 512 : (seg + 1) * 512],
                        start=True, stop=True,
                    )
                    nc.scalar.copy(
                        out=mean_stage[
                            0:1, tt * D + seg * 512 : tt * D + (seg + 1) * 512
                        ],
                        in_=mn_ps[:],
                    )

            # --- landmark projection: lm = (chunk_sums) @ wlT (wlT has /128) ---
            for tt in range(NT):
                nc.sync.dma_start(
                    out=mean_sb[tt : tt + 1, :],
                    in_=mean_stage[0:1, tt * D : (tt + 1) * D],
                )
            mt_ps = ps_m.tile([128, NT * 16], BF16, tag="misc")
            for dt in range(ND):
                nc.tensor.transpose(
                    out=mt_ps[:, dt * 16 : (dt + 1) * 16],
                    in_=mean_sb[0:NT, dt * 128 : (dt + 1) * 128],
                    identity=ident[0:NT, 0:NT],
                )
            mt_sb = lmpool.tile([128, NT * 16], BF16)
            nc.vector.tensor_copy(out=mt_sb[:], in_=mt_ps[:])
            lm_ps = ps_m.tile([NT, KV], F32, tag="misc")
            for dt in range(ND):
                nc.tensor.matmul(
                    lm_ps[:], lhsT=mt_sb[:, dt * 16 : (dt + 1) * 16],
                    rhs=wl_sb[:, dt, :],
                    start=(dt == 0), stop=(dt == ND - 1),
                )
            lm_sb = lmpool.tile([NT, KV], F32)
            nc.vector.tensor_copy(out=lm_sb[:], in_=lm_ps[:])
            nc.sync.dma_start(out=lm_d[:], in_=lm_sb[:])

    nc.finalize()
    return nc


def kernel(hidden_states, norm_weight, Wk, Wv, Wl, **_):
    global LAST_RESULTS, _NC_CACHE

    hidden_states = np.asarray(hidden_states)
    norm_weight = np.asarray(norm_weight, dtype=np.float32)

    # Fold norm_weight into the weights (exact: reference multiplies enc by w
    # elementwise over d before every projection).  Pre-transpose to [d, out]
    # so the contraction dim lands on SBUF partitions; 1/128 chunk-mean factor
    # folds into Wl.  bf16 compute dtype.
    def prep(w, scale=1.0):
        w = np.asarray(w, dtype=np.float32) * norm_weight[None, :] * scale
        return np.ascontiguousarray(w.T).astype(ml_dtypes.bfloat16)

    wkT = prep(Wk)
    wvT = prep(Wv)
    wlT = prep(Wl, scale=1.0 / 128.0)

    x_flat = hidden_states.reshape(N_BATCH * T_FULL, D)
    x_bf = x_flat.astype(ml_dtypes.bfloat16)

    if _NC_CACHE is None:
        _NC_CACHE = _build_nc()
    nc = _NC_CACHE

    in_maps = []
    for i in range(N_CORES):
        in_maps.append(
            {
                "x": np.ascontiguousarray(x_bf[i * TOK : (i + 1) * TOK]),
                "wkT": wkT,
                "wvT": wvT,
                "wlT": wlT,
            }
        )

    res = run_bass_kernel_spmd(nc, in_maps, list(range(N_CORES)))
    LAST_RESULTS = res

    mem_k = np.concatenate([r["mem_k"] for r in res.results], axis=0)
    mem_v = np.concatenate([r["mem_v"] for r in res.results], axis=0)
    lmk = np.concatenate([r["lmk"] for r in res.results], axis=0)

    mem_k = mem_k.reshape(N_BATCH, T_FULL, 8, 64)
    mem_v = mem_v.reshape(N_BATCH, T_FULL, 8, 64)
    landmarks = lmk.reshape(N_BATCH, 64, 8, 64)

    return hidden_states, mem_k, mem_v, landmarks
